# revision 43
# baseline (speedup 1.0000x reference)
"""ClassicalSelfAttention TRN2 kernel — 8-core SPMD, sequence-parallel.

out = softmax((X Wq)(X Wk)^T / sqrt(d)) @ X,  X:[4096,1024] f32, W:[1024,1024].

Per core (rows sharded 8x512):
  host:  W = Wq @ Wk^T (weight fusion — halves projection work and traffic),
         X^T f32 rotated per core so chunk 0 is the core's own column block
         (doubles as the projection rhs — no separate xlt load), X cast to
         fp8e4 twice: x8 = e4m3(X) and dx8 = e4m3(X - x8), rolled to match
         the rotated column order.
  P1:    B^T[d,m] = sum_e W[e,d] Xl^T[e,m], 64 f32r matmuls, e-ordered so
         each arriving W row-block is consumed immediately.
  S:     S[m,j] = sum_d B^T[d,m] X^T[d,j], streaming 2MB chunks, f32r
         (1 cyc/row at 512 free). Per (m,chunk): psum->SBUF copy on Act,
         running row-max on DVE. x8 halves interleave with the xt stream.
         At the last chunk each m's softmax head (rowmax, exp->fp16 with
         rowsum accum) is emitted inline so exp(m0) overlaps the chunk's
         remaining matmuls and the P3 seam starts with zero PE idle.
  P3/PV: per m: PE-transpose P to P^T (fp16 identity: 1 cyc/row), psum->SBUF
         cast to fp8 alternating DVE/Act; DoubleRow fp8 matmuls (0.5 cyc/row,
         256-contraction per instr) accumulate P8 @ x8 then P8 @ dx8 (dx8
         resident by then in the space freed by the xt stream buffers);
         double-fp8 keeps X at ~fp16 precision while the dominant weight
         (exp(0)=1.0) is exact in fp8. Scale by 1/rowsum (DVE) on the way out.

Pair schedule T0 s1(0) T1 s1(1) s2(0,1) T2 s1(2) T3 s1(3) s2(2,3) needs only
4 psum accumulator banks + transpose scratch and keeps every engine fed.
fp32r logit noise (rms ~3.4 on |S|~3e4 measured on HW) dominates rel err;
fp8 PV adds ~2e-3.
"""
import numpy as np
import ml_dtypes
import concourse.bass as bass
import concourse.bacc as bacc
import concourse.mybir as mybir
import concourse.tile as tile
from concourse import masks
from concourse.bass_utils import run_bass_kernel_spmd

F32 = mybir.dt.float32
F32R = mybir.dt.float32r
F16 = mybir.dt.float16
F8 = mybir.dt.float8e4
E4M3 = ml_dtypes.float8_e4m3
DR = mybir.MatmulPerfMode.DoubleRow
AX = mybir.AxisListType
EXP = mybir.ActivationFunctionType.Exp
COPY = mybir.ActivationFunctionType.Copy

D = 1024          # embed dim
NT = 4096         # tokens
NC = 8            # cores
NL = NT // NC     # 512 local rows
DT = D // 128     # 8 d-tiles
MT = NL // 128    # 4 m-tiles
JC = NT // 512    # 8 j-chunks
JB = NT // 256    # 16 DoubleRow j-blocks
JT = NT // 128    # 32 j-tiles
SCALE = float(1.0 / np.sqrt(np.float32(D)))


def build_nc():
    nc = bacc.Bacc("TRN2", target_bir_lowering=False, debug=False)

    w_full = nc.declare_dram_parameter("w_full", [D, D], F32, isOutput=False)
    xt_rot = nc.declare_dram_parameter("xt_rot", [D, NT], F32, isOutput=False)
    x8d = nc.declare_dram_parameter("x8d", [NT, D], F8, isOutput=False)
    dx8d = nc.declare_dram_parameter("dx8d", [NT, D], F8, isOutput=False)
    out_l = nc.declare_dram_parameter("out_local", [NL, D], F32, isOutput=True)

    with tile.TileContext(nc) as tc:
        with (
            tc.tile_pool(name="persist", bufs=1) as persist,
            tc.tile_pool(name="stats", bufs=1) as stats,
            tc.tile_pool(name="pch", bufs=2) as pch,
        ):
            identf = persist.tile([128, 128], F32, tag="idf", name="idf")
            masks.make_identity(nc, identf[:])
            ident16 = persist.tile([128, 128], F16, tag="id16", name="id16")
            nc.vector.tensor_copy(ident16[:], identf[:])
            BT = [persist.tile([128, NL], F32R, tag=f"BT{b}", name=f"BT{b}")
                  for b in range(DT)]
            pmax = [stats.tile([128, JC], F32, tag=f"pmax{m}", name=f"pmax{m}")
                    for m in range(MT)]
            recip = stats.tile([128, MT], F32, tag="recip", name="recip")
            x8 = persist.tile([128, JB, 2, D], F8, tag="x8", name="x8")
            PT8 = [persist.tile([128, JT, 128], F8, tag=f"PT{m}", name=f"PT{m}")
                   for m in range(MT)]
            pbig = []

            def softmax_head(m):
                rowmax = stats.tile([128, 1], F32, tag=f"rmx{m}", name=f"rmx{m}")
                nc.vector.reduce_max(rowmax[:], pmax[m][:], axis=AX.X)
                negb = stats.tile([128, 1], F32, tag=f"ngb{m}", name=f"ngb{m}")
                nc.vector.tensor_scalar_mul(negb[:], rowmax[:], -SCALE)
                pb16 = pch.tile([128, JC, 512], F16, tag="pbig", name=f"pbig{m}")
                esum = stats.tile([128, 2], F32, tag=f"es{m}", name=f"es{m}")
                for hf in range(2):
                    nc.scalar.activation(pb16[:, hf * 4:(hf + 1) * 4, :],
                                         S[m][:, hf * 4:(hf + 1) * 4, :], EXP,
                                         bias=negb[:], scale=SCALE,
                                         accum_out=esum[:, hf:hf + 1])
                rsum = stats.tile([128, 1], F32, tag=f"rs{m}", name=f"rs{m}")
                nc.vector.reduce_sum(rsum[:], esum[:], axis=AX.X)
                nc.vector.reciprocal(recip[:, m:m + 1], rsum[:])
                pbig.append(pb16)

            with tc.tile_pool(name="spool", bufs=1) as spool:
                S = [spool.tile([128, JC, 512], F32, tag=f"S{m}", name=f"S{m}")
                     for m in range(MT)]
                with tc.tile_pool(name="xts", bufs=3) as xts:
                    xtc0 = xts.tile([128, DT, 512], F32R, tag="xtc", name="xtc0")
                    with (
                        tc.tile_pool(name="p1", bufs=3) as p1,
                        tc.tile_pool(name="ps1", bufs=1,
                                     space=bass.MemorySpace.PSUM) as ps1,
                    ):
                        # ---- P1: stream W row-blocks through 3 buffers ----
                        pb = [ps1.tile([128, NL], F32, tag=f"pb{b}", name=f"pb{b}")
                              for b in range(DT)]
                        for _ in range(14):
                            nc.tensor.transpose(pb[0][:, 0:128], identf[:],
                                                identf[:])
                        wes = []
                        for e in range(DT):
                            nc.sync.dma_start(
                                xtc0[:, e, :],
                                xt_rot[e * 128:(e + 1) * 128, 0:512].bitcast(F32R))
                            we = p1.tile([128, D], F32R, tag="w", name=f"w{e}")
                            nc.sync.dma_start(
                                we[:], w_full[e * 128:(e + 1) * 128, :]
                                .bitcast(F32R))
                            wes.append(we)
                        for e in range(DT):
                            for b in range(DT):
                                nc.tensor.matmul(
                                    pb[b][:], wes[e][:, b * 128:(b + 1) * 128],
                                    xtc0[:, e, :], start=(e == 0),
                                    stop=(e == DT - 1))
                        for b in range(DT):
                            if b % 2 == 0:
                                nc.vector.tensor_copy(BT[b][:], pb[b][:])
                            else:
                                nc.scalar.activation(BT[b][:], pb[b][:], COPY)
                        for _ in range(16):
                            nc.tensor.transpose(pb[1][:, 0:128], identf[:],
                                                identf[:])

                    # ---- S phase ----
                    with tc.tile_pool(name="ps2", bufs=5,
                                      space=bass.MemorySpace.PSUM) as ps2:
                        for jc in range(JC):
                            if jc == 0:
                                xtc = xtc0
                            else:
                                xtc = xts.tile([128, DT, 512], F32R, tag="xtc",
                                               name=f"xtc{jc}")
                                nc.sync.dma_start(
                                    xtc[:],
                                    xt_rot[:, jc * 512:(jc + 1) * 512]
                                    .rearrange("(b p) j -> p b j", p=128)
                                    .bitcast(F32R))
                            if 1 <= jc <= 7:
                                q = jc - 1
                                nc.sync.dma_start(
                                    x8[:, q * 2:(q + 1) * 2, :, :],
                                    x8d[q * 512:(q + 1) * 512, :]
                                    .rearrange("(jb two p) d -> p jb two d",
                                               p=128, two=2))
                            for m in range(MT):
                                ps = ps2.tile([128, 512], F32, tag="ps", name="ps")
                                for b in range(DT):
                                    nc.tensor.matmul(
                                        ps[:], BT[b][:, m * 128:(m + 1) * 128],
                                        xtc[:, b, :],
                                        start=(b == 0), stop=(b == DT - 1))
                                if jc == JC - 1:
                                    nc.vector.tensor_copy(S[m][:, jc, :], ps[:])
                                else:
                                    nc.scalar.activation(S[m][:, jc, :], ps[:],
                                                         COPY)
                                nc.vector.reduce_max(pmax[m][:, jc:jc + 1], ps[:],
                                                     axis=AX.X)
                                if jc == JC - 1:
                                    softmax_head(m)

                # xts freed: last x8 quarter + resident dx8 go into its space
                with (
                    tc.tile_pool(name="dxr", bufs=1) as dxr,
                    tc.tile_pool(name="ps3", bufs=3,
                                 space=bass.MemorySpace.PSUM) as ps3,
                    tc.tile_pool(name="ps4", bufs=1,
                                 space=bass.MemorySpace.PSUM) as ps4,
                    tc.tile_pool(name="ob", bufs=2) as ob,
                ):
                    dx8 = dxr.tile([128, JB, 2, D], F8, tag="dx8", name="dx8")
                    nc.sync.dma_start(
                        dx8[:, 0:2, :, :],
                        dx8d[0:512, :]
                        .rearrange("(jb two p) d -> p jb two d", p=128, two=2))
                    nc.sync.dma_start(
                        x8[:, 14:16, :, :],
                        x8d[7 * 512:8 * 512, :]
                        .rearrange("(jb two p) d -> p jb two d", p=128, two=2))
                    for q in range(1, 8):
                        nc.sync.dma_start(
                            dx8[:, q * 2:(q + 1) * 2, :, :],
                            dx8d[q * 512:(q + 1) * 512, :]
                            .rearrange("(jb two p) d -> p jb two d", p=128, two=2))

                    accs = [[ps4.tile([128, 512], F32, tag=f"acc{mi}_{h}",
                                      name=f"acc{mi}_{h}")
                             for h in range(2)] for mi in range(2)]

                    def t_sweep1(m, acc, act_mod):
                        pbf = pbig[m][:].rearrange("p c f -> p (c f)")

                        def mm_group(g):
                            for jb in (2 * g, 2 * g + 1):
                                for h in range(2):
                                    nc.tensor.matmul(
                                        acc[h][:], PT8[m][:, 2 * jb:2 * jb + 2, :],
                                        x8[:, jb, :, h * 512:(h + 1) * 512],
                                        start=(jb == 0), stop=False, perf_mode=DR)

                        for g in range(8):
                            pt = ps3.tile([128, 4, 128], F16, tag="pt", name="pt")
                            for t in range(4):
                                jt = g * 4 + t
                                nc.tensor.transpose(
                                    pt[:, t, :], pbf[:, jt * 128:(jt + 1) * 128],
                                    ident16[:])
                            dst = PT8[m][:, g * 4:(g + 1) * 4, :]
                            if g % act_mod == act_mod - 1:
                                nc.scalar.activation(dst, pt[:], COPY)
                            else:
                                nc.vector.tensor_copy(dst, pt[:])
                            if g > 1:
                                mm_group(g - 2)
                        mm_group(6)
                        mm_group(7)

                    def out_m(m, acc):
                        osb = ob.tile([128, D], F32, tag="osb", name=f"osb{m}")
                        nc.vector.tensor_scalar_mul(
                            osb[:, 0:512], acc[0][:], recip[:, m:m + 1])
                        nc.scalar.activation(osb[:, 512:1024], acc[1][:], COPY,
                                             scale=recip[:, m:m + 1])
                        nc.sync.dma_start(out_l[m * 128:(m + 1) * 128, :], osb[:])

                    def sweep2_pair(m0, m1, a0, a1, seq):
                        if seq:
                            for m, acc in ((m0, a0), (m1, a1)):
                                for jb in range(JB):
                                    for h in range(2):
                                        nc.tensor.matmul(
                                            acc[h][:],
                                            PT8[m][:, 2 * jb:2 * jb + 2, :],
                                            dx8[:, jb, :, h * 512:(h + 1) * 512],
                                            start=False, stop=(jb == JB - 1),
                                            perf_mode=DR)
                                out_m(m, acc)
                        else:
                            for jb in range(JB):
                                last = jb == JB - 1
                                for m, acc in ((m0, a0), (m1, a1)):
                                    for h in range(2):
                                        nc.tensor.matmul(
                                            acc[h][:],
                                            PT8[m][:, 2 * jb:2 * jb + 2, :],
                                            dx8[:, jb, :, h * 512:(h + 1) * 512],
                                            start=False, stop=last, perf_mode=DR)
                                    if last:
                                        out_m(m, acc)

                    for pair in range(2):
                        m0, m1 = 2 * pair, 2 * pair + 1
                        a0, a1 = accs[0], accs[1]
                        am = 4 if pair == 0 else 2
                        t_sweep1(m0, a0, am)
                        t_sweep1(m1, a1, am)
                        sweep2_pair(m0, m1, a0, a1, seq=(pair == 1))

    nc.compile()
    return nc


_NC_CACHE = None


def kernel(inputs, rotation_params, entangle_params):
    global _NC_CACHE
    if _NC_CACHE is None:
        _NC_CACHE = build_nc()
    nc = _NC_CACHE
    x = np.ascontiguousarray(np.asarray(inputs, np.float32))
    wq = np.asarray(rotation_params, np.float32)
    wk = np.asarray(entangle_params, np.float32)
    w = np.ascontiguousarray(wq @ wk.T)
    xt = np.ascontiguousarray(x.T)
    x8_np = x.astype(E4M3)
    dx8_np = (x - x8_np.astype(np.float32)).astype(E4M3)
    in_maps = []
    for c in range(NC):
        xtr = np.ascontiguousarray(np.roll(xt, -c * NL, axis=1))
        x8r = np.ascontiguousarray(np.roll(x8_np, -c * NL, axis=0))
        dx8r = np.ascontiguousarray(np.roll(dx8_np, -c * NL, axis=0))
        in_maps.append({"w_full": w, "xt_rot": xtr, "x8d": x8r, "dx8d": dx8r})
    r = run_bass_kernel_spmd(nc, in_maps, list(range(NC)))
    return np.concatenate([r.results[c]["out_local"] for c in range(NC)], axis=0)
